# revision 127
# baseline (speedup 1.0000x reference)
"""Fused multi-head self-attention kernel for Trainium2 (Bass/Tile).

Problem: x:[4,2560,320] f32, Wq/Wk/Wv:[320,512], Wo:[512,320], bo:[320]
  q,k,v = x@W*, 8 heads x 64; sim = q k^T * d^-0.5; attn = softmax(sim);
  out = (attn @ v) @ Wo + bo.

Sharding: batch*head 32-way -> 8 cores: core c handles batch c//2 and the
4-head group c%2. Host sums the two half-head partial output projections
per batch and adds the bias.

Per-core layout trick: scores are computed TRANSPOSED (sT[j,i] = k_j . q_i)
so that the softmax denominator arrives for free: v gets a ones-column
appended, and out' = expT_slice.T @ [v|1] accumulates both attn@v and the
row sums. Normalisation is then a per-partition scalar multiply.

v6 (205us -> 169.7us on the cost-model metric). The kernel is PE-bound
(~156us of matmul rows at fp16: 205k score cols + 104k attn cols + 46k
projection cols + y) with the PSUM->SBUF drain capacity of ACT+DVE (the
only engines that can read PSUM) just underneath. Design:
  - rounds are (head, n-third) with [128,1024]-bank score tiles of width
    896/896/768: a depth-3 psum rotation takes the drain+latency off the
    critical loop (depth 2 at [128,1280] stalled the PE ~200ns/tile), and
    the near-equal widths keep the per-round drain load uniform.
  - exp split ~11 ACT (exact exp) / ~9 DVE per round; DVE tiles use a
    ONE-term Schraudolph: et = bitcast_fp16(int16(A*s + B)), one
    tensor_scalar per tile. Device-measured rel err 6.6e-3 (gate 2e-2).
    Each round's last tile drains on both engines (halves) so the next
    round's attn, which needs every et of the round, is not held up.
  - outT is built by DMA TRANSPOSES: head 2m's normalized attn output
    lands in on2[m] cols 0:64 of a block, head 2m+1 in cols 64:128, and
    one [128,128] dma_start_transpose per block writes outT for both
    heads - no PE transpose, no ACT/DVE copy (saved ~20us of drain-engine
    time + 2us PE). The last tail blocks use a PE-transpose+DVE-copy
    instead, dodging the ~2us DMA-sem latency where y-steps wait on it.
  - y-steps spread over rounds (3,1)/(3,2) (at DVE-exp slots, where the
    ACT bounce is free) and a 6-step tail; qk projection chunks spread
    one-per-slot into rounds 0, 2 and 5.
  - x DMAs on the sync HWDGE queue; weights on the gpsimd SWDGE queue.
All attention tensors fp16; accumulations f32 in PSUM.
"""

import sys

import numpy as np

if "/opt/trn_rl_repo" not in sys.path:
    sys.path.insert(0, "/opt/trn_rl_repo")

from contextlib import ExitStack

import concourse.bass as bass
from concourse import bacc
import concourse.mybir as mybir
import concourse.tile as tile
from concourse.bass_utils import run_bass_kernel_spmd
from concourse.masks import make_identity

# ---- problem constants (hardcoded per contract) ----
B = 4
N = 2560
QD = 320
H_TOT = 8
D = 64
HPC = 4                  # heads per core
IPC = HPC * D            # 256 inner dims per core
SCALE = D ** -0.5
NT = N // 128            # 20 n-tiles
HALF = N // 2            # 1280
F32 = mybir.dt.float32
F16 = mybir.dt.float16
I16 = mybir.dt.int16
EXP = mybir.ActivationFunctionType.Exp
MULT = mybir.AluOpType.mult
ADD = mybir.AluOpType.add

# One-term Schraudolph constants: exp(SCALE*s) ~ bitcast_f16(int16(A*s+B)).
# B offset calibrated end-to-end in numpy (rel err 6.3e-3 at ~45% coverage).
EXP_A1 = float(1024 * SCALE / np.log(2.0))    # 184.664
EXP_B1 = 15360.0 - 48.0

KS = [(0, 128), (128, 128), (256, 64)]       # qd=320 contraction subtiles
CHUNKS = [(0, 512), (512, 512), (1024, 256)]  # proj psum chunks (N<=512)

# rounds: (h, t): i-range IB[t] .. IB[t]+IW[t]. Near-equal widths keep the
# per-round drain load uniform (a 1024/1024/512 split left the 1024-rounds
# with a ~1.2us/round drain deficit that stalled the PE at round ends).
IB = [0, 896, 1792]
IW = [896, 896, 768]

EXP_BUFS = 52
WARM_N = 6
SBSM_BUFS = 6

# j-slots whose exp runs on DVE via the one-term bit trick (per round-width).
DVE_SLOTS_W = {896: {1, 3, 5, 7, 9, 11, 13, 15, 17},
               768: {1, 3, 5, 7, 9, 11, 13, 15, 17}}
# 768-rounds that also carry qk-projection drains on ACT keep all outT
# copies on DVE instead of one more exp tile
DVE_SLOTS_QK = {1, 3, 5, 7, 9, 11, 13, 15, 17}

# per-round overrides of the DVE exp-slot set (round 0's DVE also carries
# the v-projection copies, so it gets a lighter exp share)
ROUND_DVE = {0: {1, 3, 5, 7, 9, 11, 13},
             3: {1, 3, 5, 7, 9, 11, 13, 15, 17, 19},
             6: {1, 3, 5, 7, 9, 11, 13, 15, 17, 19},
             9: {1, 3, 5, 7, 9, 11, 13, 15, 17, 19},
             11: {1, 3, 5, 7, 9, 11, 13, 15, 17, 19}}

_built = {}
last_results = None      # stashed BassKernelResults for the test harness


def _mk_maps(ns, shift=0):
    """accum/xpose slot maps for an ns-step attn round. The normalized
    output of head 2m lands in on2[m] cols 0:64, head 2m+1 in cols 64:128;
    after the odd head's step, ONE dma-transpose of the [128,128] block
    yields the finished outT block for both heads (no PE transpose, no
    ACT/DVE copy)."""
    acc = {2 * s + 1 + shift: s for s in range(ns)}
    xps = {2 * s + 2 + shift: s for s in range(ns)}
    return acc, xps


# shift 2: the first accum otherwise races the prev round's last exp
# (which only drains ~1.5 slots into the current round)
MAPS_BY_NS = {7: _mk_maps(7, shift=2), 6: _mk_maps(6, shift=2)}
# y-slot maps: in rounds (3,1)/(3,2), y-steps for the prev round's 7
# blocks; each trails its block's dma-transpose (+900ns dma sem)
Y7 = {7: 0, 9: 1, 11: 2, 13: 3, 15: 4, 17: 5, 19: 6}


def _build():
    nc = bacc.Bacc(None, target_bir_lowering=False)
    xT = nc.declare_dram_parameter("xT", [QD, N], F16, isOutput=False)
    wq = nc.declare_dram_parameter("wq", [QD, IPC], F16, isOutput=False)
    wk = nc.declare_dram_parameter("wk", [QD, IPC], F16, isOutput=False)
    wv = nc.declare_dram_parameter("wv", [QD, IPC], F16, isOutput=False)
    wo = nc.declare_dram_parameter("wo", [IPC, QD], F16, isOutput=False)
    y = nc.declare_dram_parameter("y", [N, QD], F32, isOutput=True)

    with tile.TileContext(nc) as tc, ExitStack() as ctx:
        const = ctx.enter_context(tc.tile_pool(name="const", bufs=1))
        smps = ctx.enter_context(tc.tile_pool(name="smps", bufs=2, space="PSUM"))
        epool = ctx.enter_context(tc.tile_pool(name="epool", bufs=EXP_BUFS))
        ypool = ctx.enter_context(tc.tile_pool(name="ypool", bufs=3))
        sbsm = ctx.enter_context(tc.tile_pool(name="sbsm", bufs=SBSM_BUFS))
        spool_cm = tc.tile_pool(name="spool", bufs=3, space="PSUM")
        spool = spool_cm.__enter__()

        ident = const.tile([128, 128], F32, tag="ident", name="ident")
        make_identity(nc, ident[:])
        identh = const.tile([128, 128], F16, tag="identh", name="identh")
        make_identity(nc, identh[:])
        for _ in range(WARM_N):
            pw = smps.tile([128, 128], F32, tag="sm", name="pwarm")
            nc.tensor.matmul(pw[:], lhsT=ident[:], rhs=ident[:],
                             start=True, stop=True)

        # ---- persistent inputs (DMA emission ordered by first use) ----
        xts = [const.tile([128, N], F16, tag=f"xt{ki}", name=f"xt{ki}")
               for ki in range(3)]
        wqs = [const.tile([128, IPC], F16, tag=f"wq{ki}", name=f"wq{ki}")
               for ki in range(3)]
        wks = [const.tile([128, IPC], F16, tag=f"wk{ki}", name=f"wk{ki}")
               for ki in range(3)]
        wvs = [const.tile([128, IPC], F16, tag=f"wv{ki}", name=f"wv{ki}")
               for ki in range(3)]
        wos = [const.tile([128, QD], F16, tag=f"wo{kk}", name=f"wo{kk}")
               for kk in range(2)]
        # critical set first; x on the sync-engine HWDGE queue (ACT queue
        # stays free for exp), weights on the gpsimd SWDGE queue.
        for ki, (k0, kw) in enumerate(KS):
            nc.sync.dma_start(xts[ki][:kw, 0:512], xT[k0:k0 + kw, 0:512])
            nc.gpsimd.dma_start(wqs[ki][:kw, :], wq[k0:k0 + kw, :])
        for ki, (k0, kw) in enumerate(KS):
            nc.sync.dma_start(xts[ki][:kw, 512:1280], xT[k0:k0 + kw, 512:1280])
            nc.gpsimd.dma_start(wks[ki][:kw, :], wk[k0:k0 + kw, :])
        for cc in range(2, 4):
            for ki, (k0, kw) in enumerate(KS):
                nc.sync.dma_start(xts[ki][:kw, cc * 640:(cc + 1) * 640],
                                  xT[k0:k0 + kw, cc * 640:(cc + 1) * 640])
        for ki, (k0, kw) in enumerate(KS):
            nc.gpsimd.dma_start(wvs[ki][:kw, :], wv[k0:k0 + kw, :])
        for kk in range(2):
            nc.gpsimd.dma_start(wos[kk][:], wo[kk * 128:(kk + 1) * 128, :])
        # warm exp after the DMA dispatches so the act-table load doesn't
        # delay the x transfers
        warm = sbsm.tile([128, 1], F32, tag="warm", name="warm")
        nc.scalar.activation(warm[:], ident[:, 0:1], EXP, scale=1.0)

        # qT/kT: [inner(256) x n] as 2 tiles of [128, N] each; fp16 storage
        qk_sb = [const.tile([128, N], F16, tag=f"qk{i}", name=f"qk{i}") for i in range(4)]
        # outT: normalized attention output, [inner x n], fp16
        outT = [const.tile([128, N], F16, tag=f"oT{kk}", name=f"oT{kk}") for kk in range(2)]
        # un-transposed normalized attn outputs, [n x inner-slab] per m
        on2 = [const.tile([128, N], F16, tag=f"on{kk}", name=f"on{kk}") for kk in range(2)]
        # v with ones column per head: [n-tile][128, 4*65] fp16
        v1s = [const.tile([128, HPC * 65], F16, tag=f"v1_{j}", name=f"v1_{j}") for j in range(NT)]

        ws = [wqs, wks]
        tails = {}

        def qk_proj(ti, m, half, chunks=None):
            """qT/kT tile ti(0=q,1=k), inner slab m, col half -> qk_sb[ti*2+m]."""
            for c0, cw in (chunks or CHUNKS):
                ps = smps.tile([128, 512], F32, tag="sm", name="smp")
                for ki, (k0, kw) in enumerate(KS):
                    nc.tensor.matmul(
                        ps[:, 0:cw],
                        lhsT=ws[ti][ki][:kw, m * 128:(m + 1) * 128],
                        rhs=xts[ki][:kw, half * HALF + c0:half * HALF + c0 + cw],
                        start=(ki == 0), stop=(ki == 2),
                    )
                nc.scalar.copy(
                    qk_sb[ti * 2 + m][:, half * HALF + c0:half * HALF + c0 + cw],
                    ps[:, 0:cw])

        def v_proj(j):
            """v for n-tiles j,j+1 (all 4 heads) -> psum; copies are emitted
            separately (possibly spilled into round 1) via v_copy."""
            ps = smps.tile([128, 2 * IPC], F32, tag="sm", name="smv")
            for jj in (j, j + 1):
                off = (jj - j) * IPC
                for ki, (k0, kw) in enumerate(KS):
                    nc.tensor.matmul(
                        ps[:, off:off + IPC],
                        lhsT=xts[ki][:kw, jj * 128:(jj + 1) * 128],
                        rhs=wvs[ki][:kw, :],
                        start=(ki == 0), stop=(ki == 2),
                    )
            return ps

        def v_copy(ps, j, jj):
            off = (jj - j) * IPC
            v1v = v1s[jj][:].rearrange("p (h e) -> p h e", e=65)
            nc.gpsimd.memset(v1v[:, :, 64:65], 1.0)
            nc.vector.tensor_copy(
                v1v[:, :, 0:64],
                ps[:, off:off + IPC].rearrange("p (h d) -> p h d", d=64))

        def scores_psum(h, t, j):
            """sT[j-tile, i-third] = k_j . q_i -> PSUM f32 [128, IW[t]]."""
            m, po = h // 2, (h % 2) * 64
            ps = spool.tile([128, 1024], F32, tag="s", name="s")
            for c0 in range(0, IW[t], 512):
                cw = min(512, IW[t] - c0)
                nc.tensor.matmul(
                    ps[:, c0:c0 + cw],
                    lhsT=qk_sb[2 + m][po:po + 64, j * 128:(j + 1) * 128],
                    rhs=qk_sb[m][po:po + 64, IB[t] + c0:IB[t] + c0 + cw],
                    start=True, stop=True,
                )
            return ps

        def exp_part(et, ps, c0, cw, on_dve):
            if on_dve:
                nc.vector.tensor_scalar(et[:, c0:c0 + cw].bitcast(I16),
                                        ps[:, c0:c0 + cw],
                                        EXP_A1, EXP_B1, MULT, ADD)
            else:
                nc.scalar.activation(et[:, c0:c0 + cw], ps[:, c0:c0 + cw],
                                     EXP, scale=float(SCALE))

        def scores_exp(h, t, j, on_dve):
            """PSUM scores -> fp16 weights in SBUF (exact exp on ACT, or the
            one-term Schraudolph bit-trick on DVE)."""
            ps = scores_psum(h, t, j)
            et = epool.tile([128, 1024], F16, tag="e", name="et")
            if j == 19 and not on_dve:
                # last tile of the round: drain on BOTH engines so the next
                # round's attn (which needs every et) isn't held up
                w2 = IW[t] // 2
                exp_part(et, ps, 0, w2, on_dve=True)
                exp_part(et, ps, w2, IW[t] - w2, on_dve=False)
            else:
                exp_part(et, ps, 0, IW[t], on_dve)
            return et

        # ---- attention step machinery -------------------------------------
        def attn_accum(h, ib, ets, s, tail=False):
            """out'[step s] = sum_j expT_j.T @ [v|1]; normalize into on2."""
            m, po = h // 2, (h % 2) * 64
            tp = (tails["pool"].tile([128, 65], F32, tag="to", name="smo")
                  if tail else smps.tile([128, 65], F32, tag="sm", name="smo"))
            for j in range(NT):
                nc.tensor.matmul(
                    tp[:],
                    lhsT=ets[j][:, s * 128:(s + 1) * 128],
                    rhs=v1s[j][:, h * 65:(h + 1) * 65],
                    start=(j == 0), stop=(j == NT - 1),
                )
            rc = sbsm.tile([128, 1], F32, tag="rc", name="rc")
            nc.vector.reciprocal(rc[:], tp[:, 64:65])
            nc.vector.tensor_scalar_mul(
                on2[m][:, (ib + s) * 128 + po:(ib + s) * 128 + po + 64],
                tp[:, 0:64], rc[:])

        def attn_xpose(m, ig, fast=False):
            """one [128,128] transpose: finished outT block for both heads
            of slab m. Normally a dma transpose on the sync HWDGE queue (the
            DMA engines do the work, freeing ACT/DVE/PE); `fast` uses a PE
            transpose + DVE copy instead - higher engine cost but ~4x lower
            latency, for the last tail blocks where y-steps wait on it."""
            if fast:
                pst = tails["pool"].tile([128, 64], F32, tag="tx", name="pstx")
                nc.tensor.transpose(pst[:].bitcast(F16),
                                    on2[m][:, ig * 128:(ig + 1) * 128],
                                    identh[:])
                nc.vector.tensor_copy(outT[m][:, ig * 128:(ig + 1) * 128],
                                      pst[:].bitcast(F16))
            else:
                nc.sync.dma_start_transpose(
                    outT[m][:, ig * 128:(ig + 1) * 128],
                    on2[m][:, ig * 128:(ig + 1) * 128])

        def y_step(i, tail=False, on_act=True):
            """y[i-tile] = outT[:, i].T @ Wo -> DRAM (SBUF bounce)."""
            psy = (tails["pool"].tile([128, QD], F32, tag="ty", name="smy")
                   if tail else smps.tile([128, QD], F32, tag="sm", name="smy"))
            for kk in range(2):
                nc.tensor.matmul(
                    psy[:],
                    lhsT=outT[kk][:, i * 128:(i + 1) * 128],
                    rhs=wos[kk][:],
                    start=(kk == 0), stop=(kk == 1),
                )
            ysb = ypool.tile([128, QD], F32, tag="y", name="ysb")
            if on_act:
                nc.scalar.copy(ysb[:], psy[:])
            else:
                nc.vector.tensor_copy(ysb[:], psy[:])
            nc.sync.dma_start(y[i * 128:i * 128 + 64, :], ysb[0:64, :])
            nc.gpsimd.dma_start(y[i * 128 + 64:(i + 1) * 128, :], ysb[64:128, :])

        # ---- emission ----
        # upfront: q m0 h0 and k m0 h0 (needed from round 0 slot 0)
        qk_proj(0, 0, 0)
        qk_proj(1, 0, 0, chunks=CHUNKS[:1])
        qk_proj(1, 0, 0, chunks=CHUNKS[1:])
        # remaining projections, spread one chunk per slot:
        #   round 0: k m0 h1 (slots 1-3, needed by slot 10) and q m0 h1
        #            (slots 5-7, needed by round 1)
        #   round 2 (light): k m1 h0+h1; round 5 (light): q m1 h0+h1
        pending = {}
        for (rr, j0, stp), pr in [((0, 1, 1), (1, 0, 1)), ((0, 5, 1), (0, 0, 1)),
                                  ((2, 2, 3), (1, 1, 0)), ((2, 12, 3), (1, 1, 1)),
                                  ((5, 2, 3), (0, 1, 0)), ((5, 12, 3), (0, 1, 1))]:
            for ci in range(3):
                pending[(rr, j0 + ci * stp)] = pr + (ci,)

        # v copies spilled from round 0 into round 1: slot -> (ps, j, jj)
        vspill = {}

        prev = None          # (h, ib_tiles, ets, nsteps)
        for r in range(12):
            h, t = divmod(r, 3)
            if r in ROUND_DVE:
                dve_slots = ROUND_DVE[r]
            elif r in (2, 5):
                dve_slots = DVE_SLOTS_QK
            else:
                dve_slots = DVE_SLOTS_W[IW[t]]
            ets = []
            if prev is None:
                amap, xmap = {}, {}
            else:
                amap, xmap = MAPS_BY_NS[prev[3]]
                if prev[0] % 2 == 0:
                    xmap = {}      # transposes fire after the odd head
            ymap = Y7 if r in (10, 11) else {}
            yb = (r - 10) * 7 if r in (10, 11) else 0
            for j in range(NT):
                ets.append(scores_exp(h, t, j, j in dve_slots))

                pr = pending.pop((r, j), None)
                if pr is not None:
                    ti, m, hf, ci = pr
                    qk_proj(ti, m, hf, chunks=CHUNKS[ci:ci + 1])
                if r == 0 and j % 2 == 0:
                    ps = v_proj(j)
                    v_copy(ps, j, j)
                    if j < 16:
                        v_copy(ps, j, j + 1)
                    else:
                        vspill[j - 16] = (ps, j, j + 1)
                elif prev is not None:
                    ph, pib, pets, pns = prev
                    sp = vspill.pop(j, None)
                    if sp is not None:
                        v_copy(*sp)
                    ai = amap.get(j)
                    if ai is not None:
                        attn_accum(ph, pib, pets, ai)
                    xs = xmap.get(j)
                    if xs is not None:
                        attn_xpose(ph // 2, pib + xs)
                    yi = ymap.get(j)
                    if yi is not None:
                        y_step(yb + yi, on_act=(yi % 2 == 0))
            prev = (h, IB[t] // 128, ets, IW[t] // 128)
        spool_cm.__exit__(None, None, None)
        tpool = ctx.enter_context(tc.tile_pool(name="tpool", bufs=2, space="PSUM"))
        tails["pool"] = tpool
        # tail: attn of round (3,2) (6 steps) + y for blocks 14..19; the
        # last two blocks transpose via PE+DVE (low latency) since the
        # final y-steps would otherwise idle on the DMA-transpose sem
        ph, pib, pets, pns = prev
        for s in range(6):
            attn_accum(ph, pib, pets, s, tail=True)
            if s >= 2:
                y_step(pib + s - 2, tail=True)
            if s >= 1:
                attn_xpose(ph // 2, pib + s - 1, fast=True)
        y_step(pib + 4, tail=True)
        attn_xpose(ph // 2, pib + 5, fast=True)
        y_step(pib + 5, tail=True)

    nc.compile()
    return nc


def _get_nc():
    if "nc" not in _built:
        _built["nc"] = _build()
    return _built["nc"]


def kernel(x, Wq, Wk, Wv, Wo, bo):
    global last_results
    x = np.asarray(x, dtype=np.float32)
    Wq = np.asarray(Wq, dtype=np.float32)
    Wk = np.asarray(Wk, dtype=np.float32)
    Wv = np.asarray(Wv, dtype=np.float32)
    Wo = np.asarray(Wo, dtype=np.float32)
    bo = np.asarray(bo, dtype=np.float32)

    nc = _get_nc()
    in_maps = []
    for c in range(8):
        bb, g = divmod(c, 2)
        sl = slice(g * IPC, (g + 1) * IPC)
        in_maps.append({
            "xT": np.ascontiguousarray(x[bb].T.astype(np.float16)),
            "wq": np.ascontiguousarray(Wq[:, sl].astype(np.float16)),
            "wk": np.ascontiguousarray(Wk[:, sl].astype(np.float16)),
            "wv": np.ascontiguousarray(Wv[:, sl].astype(np.float16)),
            "wo": np.ascontiguousarray(Wo[sl, :].astype(np.float16)),
        })
    res = run_bass_kernel_spmd(nc, in_maps, core_ids=list(range(8)))
    last_results = res
    parts = [r["y"] for r in res.results]
    out = np.empty((B, N, QD), dtype=np.float32)
    for bb in range(B):
        out[bb] = parts[2 * bb] + parts[2 * bb + 1]
    out += bo
    return out
